# revision 2
# baseline (speedup 1.0000x reference)
"""Pairwise squared-distance kernel for Trainium2 (8 NeuronCores).

out[i, j] = mean_d (x_i[d] - y_j[d])^2
          = (||x_i||^2 + ||y_j||^2 - 2 x_i . y_j) / D

Sharding: rows of z_queries split across 8 cores (1024 rows each);
class_prototypes replicated. Each core computes its [1024, 4096] slab.

Device kernel (per core):
  - inputs pre-transposed on host to [D, rows] so the contraction dim is
    the SBUF partition dim (clean contiguous DMAs, no on-chip transpose).
  - prototypes pre-scaled by -2/D (= -2^-8, exact) so PSUM accumulates
    -2/D * x.y directly.
  - GEMM: for each (m-tile 128 queries, n-half 2048 protos): 4 k-tiles x
    4 n-subtiles of matmul into a [128, 2048] PSUM tile (4 banks).
  - epilogue: one DVE scalar_tensor_tensor: (psum + xsq/D[i]) + ysq/D[j].
  - 1 MiB output DMAs.
"""

import sys

if "/opt/trn_rl_repo" not in sys.path:
    sys.path.insert(0, "/opt/trn_rl_repo")

import numpy as np

N_CORES = 8
N_Q = 8192
N_P = 4096
D = 512
ROWS = N_Q // N_CORES  # 1024 query rows per core

P = 128
M_TILES = ROWS // P  # 8
K_TILES = D // P  # 4
N_PSUM = 2048  # psum tile free dim (4 banks of fp32)
N_HALVES = N_P // N_PSUM  # 2
NB = 512  # matmul free dim (1 psum bank)
NSUB = N_PSUM // NB  # 4

# "bf16" halves input DMA traffic; "f32r" keeps fp32 inputs at TF32 matmul rate.
COMPUTE_DT = "bf16"

_CACHE = {}


def _build_nc(compute_dt: str):
    import concourse.mybir as mybir
    import concourse.tile as tile
    from concourse import bacc

    if compute_dt == "bf16":
        in_dt = mybir.dt.bfloat16
        mm_cast = lambda ap: ap
    elif compute_dt == "f32r":
        in_dt = mybir.dt.float32
        mm_cast = lambda ap: ap.bitcast(mybir.dt.float32r)
    else:
        raise ValueError(compute_dt)

    f32 = mybir.dt.float32
    add = mybir.AluOpType.add

    nc = bacc.Bacc("TRN2", target_bir_lowering=False, debug=False, num_devices=N_CORES)

    qt = nc.dram_tensor("qt", (D, ROWS), in_dt, kind="ExternalInput")
    pt = nc.dram_tensor("pt", (D, N_P), in_dt, kind="ExternalInput")
    ab = nc.dram_tensor("ab", (P, M_TILES), f32, kind="ExternalInput")
    bb = nc.dram_tensor("bb", (P, N_P), f32, kind="ExternalInput")
    out = nc.dram_tensor("out", (ROWS, N_P), f32, kind="ExternalOutput")

    with tile.TileContext(nc) as tc:
        with (
            tc.tile_pool(name="inputs", bufs=1) as in_pool,
            tc.tile_pool(name="outs", bufs=3) as out_pool,
            tc.tile_pool(name="psum", bufs=2, space="PSUM") as psum_pool,
        ):
            # Load everything once; it all fits in SBUF.
            qt_tiles = []
            for k in range(K_TILES):
                qt_t = in_pool.tile([P, ROWS], in_dt, name=f"qt_{k}")
                nc.sync.dma_start(out=qt_t, in_=qt[k * P : (k + 1) * P, :])
                qt_tiles.append(qt_t)

            # pt chunks per (k, half); half-A chunks first so compute can start
            # before half-B arrives.
            pt_tiles = [[None] * K_TILES for _ in range(N_HALVES)]
            for h in range(N_HALVES):
                for k in range(K_TILES):
                    pt_t = in_pool.tile([P, N_PSUM], in_dt, name=f"pt_{h}_{k}")
                    nc.sync.dma_start(
                        out=pt_t,
                        in_=pt[k * P : (k + 1) * P, h * N_PSUM : (h + 1) * N_PSUM],
                    )
                    pt_tiles[h][k] = pt_t
                if h == 0:
                    ab_t = in_pool.tile([P, M_TILES], f32, name="ab_t")
                    nc.sync.dma_start(out=ab_t, in_=ab[:, :])
                    bb_t = in_pool.tile([P, N_P], f32, name="bb_t")
                    nc.sync.dma_start(out=bb_t, in_=bb[:, :])

            for h in range(N_HALVES):
                for m in range(M_TILES):
                    psum_t = psum_pool.tile([P, N_PSUM], f32, name="psum_t")
                    for k in range(K_TILES):
                        lhsT = mm_cast(qt_tiles[k][:, m * P : (m + 1) * P])
                        for ns in range(NSUB):
                            nc.tensor.matmul(
                                psum_t[:, ns * NB : (ns + 1) * NB],
                                lhsT,
                                mm_cast(pt_tiles[h][k][:, ns * NB : (ns + 1) * NB]),
                                start=(k == 0),
                                stop=(k == K_TILES - 1),
                            )
                    out_t = out_pool.tile([P, N_PSUM], f32, name="out_t")
                    # out = (psum + xsq/D[i]) + ysq/D[j]
                    nc.vector.scalar_tensor_tensor(
                        out=out_t,
                        in0=psum_t,
                        scalar=ab_t[:, m : m + 1],
                        in1=bb_t[:, h * N_PSUM : (h + 1) * N_PSUM],
                        op0=add,
                        op1=add,
                    )
                    nc.sync.dma_start(
                        out=out[m * P : (m + 1) * P, h * N_PSUM : (h + 1) * N_PSUM],
                        in_=out_t,
                    )

    nc.compile()
    return nc


def _get_nc(compute_dt: str):
    if compute_dt not in _CACHE:
        _CACHE[compute_dt] = _build_nc(compute_dt)
    return _CACHE[compute_dt]


def _prep_inputs(z_queries: np.ndarray, class_prototypes: np.ndarray, compute_dt: str):
    import ml_dtypes

    np_in = ml_dtypes.bfloat16 if compute_dt == "bf16" else np.float32

    z = np.ascontiguousarray(z_queries, dtype=np.float32)
    p = np.ascontiguousarray(class_prototypes, dtype=np.float32)

    a = (z.astype(np.float64) ** 2).sum(axis=1) / D  # (N_Q,) ||x||^2 / D
    b = (p.astype(np.float64) ** 2).sum(axis=1) / D  # (N_P,) ||y||^2 / D

    pt = np.ascontiguousarray(p.T * np.float32(-2.0 / D)).astype(np_in)  # [D, N_P]
    bb = np.ascontiguousarray(
        np.broadcast_to(b.astype(np.float32), (P, N_P))
    )  # [P, N_P]

    in_maps = []
    for c in range(N_CORES):
        sl = slice(c * ROWS, (c + 1) * ROWS)
        qt_c = np.ascontiguousarray(z[sl].T).astype(np_in)  # [D, ROWS]
        ab_c = np.ascontiguousarray(
            a[sl].astype(np.float32).reshape(M_TILES, P).T
        )  # [P, M_TILES]
        in_maps.append({"qt": qt_c, "pt": pt, "ab": ab_c, "bb": bb})
    return in_maps


def run(z_queries, class_prototypes, compute_dt=COMPUTE_DT, **spmd_kwargs):
    from concourse.bass_utils import run_bass_kernel_spmd

    nc = _get_nc(compute_dt)
    in_maps = _prep_inputs(z_queries, class_prototypes, compute_dt)
    res = run_bass_kernel_spmd(nc, in_maps, core_ids=list(range(N_CORES)), **spmd_kwargs)
    full = np.concatenate([r["out"] for r in res.results], axis=0)
    return full, res


def kernel(z_queries: np.ndarray, class_prototypes: np.ndarray) -> np.ndarray:
    full, _ = run(z_queries, class_prototypes)
    return full
